# revision 2
# baseline (speedup 1.0000x reference)
"""CrissCross(actually full)-attention Trainium2 kernel, v4.

Reference computation per batch b (C=64 channels, HW=4096 positions, D=8):
    q = Wq@x + bq        [D, HW]
    k = Wk@x + bk        [D, HW]
    v = Wv@x + bv        [C, HW]
    att[i, j] = softmax_i(q[:, i] . k[:, j])
    out[c, j] = sum_i v[c, i] att[i, j] + x[c, j]

Sharding: data-parallel, one batch per NeuronCore (8 cores).

Structure (hardware-A/B-tuned):
- The softmax exp (16.7M entries/core, the ScalarE floor at 1 elem/lane/
  cycle) is split ~22/10 per j-tile between ScalarE (true exp, pair calls
  of [128,1024] PSUM->SBUF bf16) and VectorE (Schraudolph fast-exp: one
  fused tensor_scalar int16(s*2^7/ln2 + 127*128-7) whose bits are the
  bf16 exp(s) to within +-3%; softmax renormalization cancels the common
  factor -> ~6.5e-3 end-to-end rel err vs the 2e-2 gate).
- QK matmuls run f32r, 4-way row-tiled at partition groups {0,32,64,96}
  (2-way for the first two j-tiles while replica scatters land).
- AV accumulates TRANSPOSED: out^T[j,c] with vt [128,65] bf16 as the
  65-wide moving operand and the att block as stationary (~2x cheaper on
  the PE than the [65,512] orientation); the four j-block accumulators of
  a j-tile pack into ONE PSUM bank (one start zeroes the whole 2KB zero
  region, one stop closes it). The vT ones-column emits the softmax
  denominator as accumulator column 64.
- Epilogue per j-tile: per-partition reciprocal of the denominator
  column, one broadcast tensor_tensor normalize, four PE transposes (via
  an iota-built identity) accumulated into one PSUM bank, one fused
  scalar_tensor_tensor (+bv per-partition, +x residual), one out DMA.
  bv cancels out of the softmax-weighted average so it folds into the
  epilogue; bq/bk fold into the projection-evacuation tensor_scalar.
- Startup: exp-table warm-up activation at t=0, weights on HWDGE, x on
  SWDGE, chunk-0 q/k unloads via DVE copies, replica tails batched on
  the idle SWDGE queues.

PSUM: qkA 2x[128,1024] (4 banks) + qkD 2x[128,512] (2) + av/tr 2x1 = 8.
"""

import numpy as np

import bass_rust
import concourse.bass as bass
import concourse.tile as tile
from concourse import mybir
from concourse.bass_utils import run_bass_kernel_spmd

B, C, HW, D = 8, 64, 4096, 8
H = W = 64
JT = 512          # j-tile width (PSUM bank)
NJ = HW // JT     # 8
IB = 128          # i-block height (partitions)
NI = HW // IB     # 32
QG = 4            # i-blocks per QK emission group
NG = NI // QG     # 8 groups per j-tile

F32 = mybir.dt.float32
F32R = mybir.dt.float32r
BF16 = mybir.dt.bfloat16
I16 = mybir.dt.int16

# Schraudolph fast-exp in bf16: exp(x) ~= bitcast_bf16(int16(x*EA + EB)) —
# int16(x*2^7/ln2 + 127*2^7 - C) is the bf16 bit pattern of exp(x) to
# within +-3% (the linear-mantissa sawtooth; C centers it around 1).
EA = float(2.0 ** 7 / np.log(2.0))
EB = float(127.0 * 128.0 - 7.0)

# group templates per j-tile: 'X' = 4 blocks to ScalarE (2 pair calls),
# 'Y' = 2 blocks to ScalarE (1 pair) + 2 blocks to VectorE fast-exp.
PAT_JT0 = "XXXXXXXX"
PAT_STEADY = "XYYXYYXY"


def _fix_drain_waits(nc):
    """walrus in this container rejects instructions carrying more than one
    sync-wait; hoist extras onto NoOps inserted just before, same engine."""
    for f in nc.m.functions:
        for blk in f.blocks:
            insts = blk.instructions
            for tgt in [
                i for i in list(insts)
                if i.sync_info and len(i.sync_info.on_wait or []) > 1
            ]:
                si = tgt.sync_info
                waits = list(si.on_wait)
                si.on_wait = waits[-1:]
                di = insts.index(tgt)
                for w in waits[:-1]:
                    n = nc.engines[tgt.engine].nop()
                    for b in f.blocks:
                        bi = b.instructions
                        for idx in range(len(bi) - 1, -1, -1):
                            if bi[idx].name == n.ins.name:
                                bi.pop(idx)
                                break
                    n.ins.sync_info = bass_rust.SyncInfo(on_wait=[w], on_update=[])
                    insts.insert(di, n.ins)
                    di += 1


def build_nc(loop_n=None, bodies=1):
    nc = bass.Bass()
    x_d = nc.dram_tensor("x", [C, HW], F32, kind="ExternalInput")
    wq_d = nc.dram_tensor("Wq", [D, C], F32, kind="ExternalInput")
    bq_d = nc.dram_tensor("bq", [D], F32, kind="ExternalInput")
    wk_d = nc.dram_tensor("Wk", [D, C], F32, kind="ExternalInput")
    bk_d = nc.dram_tensor("bk", [D], F32, kind="ExternalInput")
    wv_d = nc.dram_tensor("Wv", [C, C], F32, kind="ExternalInput")
    bv_d = nc.dram_tensor("bv", [C], F32, kind="ExternalInput")
    out_d = nc.dram_tensor("out", [C, HW], F32, kind="ExternalOutput")

    with tile.TileContext(nc) as tc:
        with (
            tc.tile_pool(name="const", bufs=1) as cp,
            tc.tile_pool(name="qtmp", bufs=2) as qp,
            tc.tile_pool(name="attA", bufs=3) as aap,
            tc.tile_pool(name="attD", bufs=3) as adp,
            tc.tile_pool(name="epi", bufs=2) as ep,
            tc.tile_pool(name="otp", bufs=8) as otp,
            tc.tile_pool(name="psA", bufs=2, space="PSUM") as ppA,
            tc.tile_pool(name="psD", bufs=2, space="PSUM") as ppD,
            tc.tile_pool(name="psV", bufs=2, space="PSUM") as ppV,
        ):
            # ---- persistent SBUF tensors ----
            # biases are NOT folded into the projections via an ones row:
            # bq/bk are added per-partition by the projection-evacuation
            # tensor_scalar, and bv cancels out of the softmax-weighted
            # average (sum_i v p / Z = sum_i (Wv x) p / Z + bv) so it is
            # added in the epilogue instead.
            x_raw = cp.tile([C, HW], F32, tag="xraw")        # residual source
            x_sb = cp.tile([C, HW], F32R, tag="x")
            w_raw = cp.tile([C, 2 * D + C], F32, tag="wraw")
            # [WqT | 0 | WkT] with k's columns at 32-39 (PSUM rows 32-39)
            wqk_sb = cp.tile([C, 32 + D], F32R, tag="wqk")
            wv_sb = cp.tile([C, C], F32R, tag="wv")          # WvT
            bqk_col = cp.tile([32 + D, 1], F32, tag="bqk")   # bq @0-7, bk @32-39
            bv_col = cp.tile([C, 1], F32, tag="bvcol")
            q_sb = cp.tile([96 + D, HW], F32R, tag="q")      # replicas @0/32/64/96
            k_sb = cp.tile([96 + D, HW], F32R, tag="k")
            vt_sb = cp.tile([IB, NI, C + 1], BF16, tag="vt") # vT' blocks
            ones_sb = cp.tile([1, 1], F32, tag="ones")
            ident_i = cp.tile([IB, IB], mybir.dt.int32, tag="identi")
            ident = cp.tile([IB, IB], F32, tag="ident")      # transpose ifmap
            warm_sb = cp.tile([1, 1], F32, tag="warm")

            # ---- preamble ----
            # ACT exp-table warm-up: the one-time table load overlaps the
            # x DMA instead of stalling the first real exp.
            nc.vector.memset(ones_sb[:, :], 1.0)
            nc.vector.memset(bqk_col[:, :], 0.0)
            nc.scalar.activation(warm_sb[0:1, 0:1], ones_sb[0:1, 0:1],
                                 mybir.ActivationFunctionType.Exp)
            # identity ifmap for the PE-transpose epilogue: iota(f - p) == 0
            nc.gpsimd.iota(ident_i[:, :], pattern=[[1, IB]], base=0,
                           channel_multiplier=-1)
            nc.vector.tensor_scalar(ident[:, :], ident_i[:, :], 0, None,
                                    op0=mybir.AluOpType.is_equal)
            # x chunk 0 on the SWDGE (gpsimd) queue, first 512 split off so
            # the projection chain starts ~2.6us in; the small weight loads
            # go on HWDGE where descriptor gen is faster.
            nc.gpsimd.dma_start(out=x_raw[:, 0:JT], in_=x_d[:, 0:JT])
            nc.gpsimd.dma_start(out=x_raw[:, JT:1024], in_=x_d[:, JT:1024])
            nc.sync.dma_start(out=w_raw[:, 0:D], in_=wq_d.rearrange("d c -> c d"))
            nc.sync.dma_start(out=bqk_col[0:D, 0:1], in_=bq_d[:, None])
            nc.sync.dma_start(out=w_raw[:, D:2 * D], in_=wk_d.rearrange("d c -> c d"))
            nc.sync.dma_start(out=bqk_col[32:32 + D, 0:1], in_=bk_d[:, None])
            nc.sync.dma_start(out=w_raw[:, 2 * D:], in_=wv_d.rearrange("o c -> c o"))
            nc.sync.dma_start(out=bv_col[:, 0:1], in_=bv_d[:, None])
            # vT ones column, ch-0 x round, wqk assembly (DVE)
            nc.vector.memset(vt_sb[:, :, C:C + 1], 1.0)
            nc.vector.memset(wqk_sb[:, :].bitcast(F32), 0.0)
            nc.vector.tensor_copy(x_sb[:, 0:JT], x_raw[:, 0:JT])
            nc.vector.tensor_copy(wqk_sb[:, 0:D], w_raw[:, 0:D])
            nc.vector.tensor_copy(wqk_sb[:, 32:32 + D], w_raw[:, D:2 * D])
            nc.vector.tensor_copy(x_sb[:, JT:1024], x_raw[:, JT:1024])

            x_state = [True, False, False, False]
            x_rounded = [True, False, False, False]
            # SWDGE DMAs inside a For_i loop need InstIncSwdgeSem, which
            # this walrus can't codegen — the timing build (loop_n set)
            # routes in-loop DMAs through HWDGE instead.
            dq = nc.sync if loop_n else nc.gpsimd

            def ensure_x(ch):
                cs = slice(ch * 1024, (ch + 1) * 1024)
                if not x_state[ch]:
                    x_state[ch] = True
                    dq.dma_start(out=x_raw[:, cs], in_=x_d[:, cs])
                if not x_rounded[ch]:
                    x_rounded[ch] = True
                    nc.vector.tensor_copy(x_sb[:, cs], x_raw[:, cs])

            wv_copied = [False]

            def ensure_wv():
                if not wv_copied[0]:
                    wv_copied[0] = True
                    nc.vector.tensor_copy(wv_sb[:, :], w_raw[:, 2 * D:])

            def _compute():
                # qk projections: chunks ct (512 wide) in pairs sharing a
                # [40, 1024] tmp tile; the pair unload covers rows 0-7 and
                # the row-32 replicas in 3 DMAs.
                tmp_pair = [None]

                def emit_qk_proj(ct):
                    ensure_x(ct // 2)
                    js = slice(ct * JT, (ct + 1) * JT)
                    pqk = ppD.tile([32 + D, JT], F32, tag="pd")
                    nc.tensor.matmul(pqk[:, :], lhsT=wqk_sb[:, :],
                                     rhs=x_sb[:, js], start=True, stop=True)
                    if ct % 2 == 0:
                        tmp_pair[0] = qp.tile([32 + D, 2 * JT], F32R,
                                              tag="qktmp", name="qktmp")
                    tmp = tmp_pair[0]
                    half = (ct % 2) * JT
                    # evacuate + add bq/bk per-partition + round to f32r
                    nc.vector.tensor_scalar(
                        tmp[:, half:half + JT], pqk[:, :], bqk_col[:, 0:1],
                        None, op0=mybir.AluOpType.add)
                    if ct == 0:
                        # chunk 0 gates the first QK group: unload with DVE
                        # copies (32-aligned partition shifts), faster than
                        # queueing DMAs behind the weight loads on HWDGE
                        nc.vector.tensor_copy(q_sb[0:D, js], tmp[0:D, 0:JT])
                        nc.vector.tensor_copy(q_sb[32:32 + D, js],
                                              tmp[0:D, 0:JT])
                        nc.vector.tensor_copy(k_sb[0:D, js],
                                              tmp[32:32 + D, 0:JT])
                        nc.vector.tensor_copy(k_sb[32:32 + D, js],
                                              tmp[32:32 + D, 0:JT])
                    elif ct == 1:
                        # chunk 1 alone (chunk 0 went via DVE); row-32 k
                        # replica too (j-tile 1 reads k chunk 1)
                        nc.sync.dma_start(out=q_sb[0:D, js],
                                          in_=tmp[0:D, JT:])
                        nc.sync.dma_start(out=q_sb[32:32 + D, js],
                                          in_=tmp[0:D, JT:])
                        nc.sync.dma_start(out=k_sb[0:D, js],
                                          in_=tmp[32:32 + D, JT:])
                        nc.sync.dma_start(out=k_sb[32:32 + D, js],
                                          in_=tmp[32:32 + D, JT:])
                    elif ct % 2 == 1:
                        bs = slice((ct - 1) * JT, (ct + 1) * JT)
                        nc.sync.dma_start(out=q_sb[0:D, bs], in_=tmp[0:D, :])
                        nc.sync.dma_start(out=q_sb[32:32 + D, bs], in_=tmp[0:D, :])
                        nc.sync.dma_start(out=k_sb[0:D, bs],
                                          in_=tmp[32:32 + D, :])

                def emit_vt_proj(g):
                    ensure_x((g * QG * IB) // 1024)
                    ensure_x(((g + 1) * QG * IB - 1) // 1024)
                    ensure_wv()
                    pv = ppD.tile([IB, QG * C], F32, tag="pd")
                    for u in range(QG):
                        ib = g * QG + u
                        isl = slice(ib * IB, (ib + 1) * IB)
                        nc.tensor.matmul(pv[:, u * C:(u + 1) * C],
                                         lhsT=x_sb[:, isl], rhs=wv_sb[:, :],
                                         start=True, stop=True)
                    nc.vector.tensor_copy(
                        vt_sb[:, g * QG:(g + 1) * QG, 0:C],
                        pv[:, :].rearrange("p (v c) -> p v c", v=QG))

                pend_av = []     # (ib, att_tile, col0)
                pend_epi = []    # [av4, j-slice, state]

                def flush_av(av4):
                    # transposed AV: out^T[j,c] accumulates with vt as the
                    # 65-wide moving operand and the att block stationary —
                    # ~27ns/MM instead of 213 for the [65,512] orientation.
                    # av4 packs the four j-block accumulators of this j-tile
                    # side by side in one PSUM bank.
                    # one start zeroes the whole 2KB bank (all 4 packed
                    # accumulators — the PSUM zero region is the bank), one
                    # stop on the last matmul closes the group.
                    for ib, att, col0 in pend_av:
                        for u in range(QG):
                            nc.tensor.matmul(
                                av4[:, u, 0:C + 1],
                                lhsT=att[:, col0 + u * IB:col0 + (u + 1) * IB],
                                rhs=vt_sb[:, ib, :],
                                start=(ib == 0 and u == 0),
                                stop=(ib == NI - 1 and u == QG - 1))
                    pend_av.clear()

                def emit_epilogue_step(step):
                    if not pend_epi:
                        return
                    av4, pjs, state = pend_epi[0]
                    if step == 0 and "ot" not in state:
                        # 1/den per j (per-partition!), then normalize all
                        # four j-blocks in one broadcast tensor_tensor
                        recips = ep.tile([IB, QG], F32, tag="recips")
                        nc.vector.reciprocal(recips[:, :],
                                             av4[:, :, C:C + 1])
                        ot = otp.tile([IB, QG, C], F32, tag="ot", name="ot")
                        nc.vector.tensor_tensor(
                            ot[:, :, :], av4[:, :, 0:C],
                            recips[:, :].to_broadcast([IB, QG, C]),
                            op=mybir.AluOpType.mult)
                        state["ot"] = ot
                    elif step == 1:
                        if "ot" not in state:
                            emit_epilogue_step(0)
                        ot = state["ot"]
                        # 4 transposes accumulate into one PSUM bank (one
                        # start/stop pair — the bank is one zero region),
                        # then a single fused residual+bias op and one DMA
                        tr4 = ppD.tile([C, QG, IB], F32, tag="pd", name="tr4")
                        for u in range(QG):
                            nc.tensor.matmul(
                                tr4[:, u, :], lhsT=ot[:, u, :],
                                rhs=ident[:, :], is_transpose=True,
                                start=(u == 0), stop=(u == QG - 1))
                        o = ep.tile([C, JT], F32, tag="o")
                        nc.vector.scalar_tensor_tensor(
                            o[:, :], tr4[:, :, :].rearrange("c u j -> c (u j)"),
                            bv_col[:, 0:1], x_raw[:, pjs],
                            op0=mybir.AluOpType.add,
                            op1=mybir.AluOpType.add)
                        nc.sync.dma_start(out=out_d[:, pjs], in_=o[:, :])
                        pend_epi.pop(0)

                # projections for chunks 0-1 ahead of the first QK group
                emit_qk_proj(0)
                emit_qk_proj(1)

                for jt in range(NJ):
                    js = slice(jt * JT, (jt + 1) * JT)
                    pat = PAT_JT0 if jt == 0 else PAT_STEADY
                    av4 = ppV.tile([IB, QG, IB], F32, tag="av")
                    for g in range(NG):
                        gtype = pat[g]
                        b0 = g * QG
                        # row-tiling: 2-way until the 64/96 replicas land
                        nrep = 2 if jt < 2 else 4
                        qslices = [slice((b0 + u) * IB, (b0 + u + 1) * IB)
                                   for u in range(QG)]
                        rows = [32 * (u % nrep) for u in range(QG)]
                        if gtype == "X":
                            qkt = [ppA.tile([IB, 2 * JT], F32, tag="qa",
                                            name="qa") for _ in range(2)]
                            dests = [(qkt[0], 0), (qkt[0], JT),
                                     (qkt[1], 0), (qkt[1], JT)]
                        else:
                            qkt = [ppA.tile([IB, 2 * JT], F32, tag="qa",
                                            name="qa")]
                            qd = [ppD.tile([IB, JT], F32, tag="pd", name="qd")
                                  for _ in range(2)]
                            dests = [(qkt[0], 0), (qkt[0], JT),
                                     (qd[0], 0), (qd[1], 0)]
                        for u in range(QG):
                            t, col0 = dests[u]
                            nc.tensor.matmul(
                                t[:, col0:col0 + JT],
                                lhsT=q_sb[rows[u]:rows[u] + D, qslices[u]],
                                rhs=k_sb[rows[u]:rows[u] + D, js],
                                start=True, stop=True,
                                tile_position=(rows[u], 0))
                        # exp emissions
                        new_av = []
                        for pi, t in enumerate(qkt):
                            attA = aap.tile([IB, 2 * JT], BF16, tag="attA")
                            nc.scalar.activation(
                                attA[:, :], t[:, :],
                                mybir.ActivationFunctionType.Exp)
                            new_av.append((b0 + 2 * pi, attA, 0))
                            new_av.append((b0 + 2 * pi + 1, attA, JT))
                        if gtype == "Y":
                            for di, qdt in enumerate(qd):
                                attD = adp.tile([IB, JT], BF16, tag="attD")
                                nc.vector.tensor_scalar(
                                    attD[:, :].bitcast(I16), qdt[:, :],
                                    EA, EB,
                                    op0=mybir.AluOpType.mult,
                                    op1=mybir.AluOpType.add)
                                new_av.append((b0 + 2 + di, attD, 0))
                        # j-tile 0: interleave projections + tail scatters
                        if jt == 0:
                            if g in (1, 3, 5) and g + 2 < NG + 1:
                                emit_qk_proj(g + 1)
                                emit_qk_proj(g + 2)
                            elif g == 7:
                                # replica tails on the idle SWDGE queues:
                                # q rows 64/96 (all chunks); k rows 32 for
                                # chunks 2-7 and 64/96 for chunks 1-7
                                for r in (2, 3):
                                    dq.dma_start(
                                        out=q_sb[32 * r:32 * r + D, :],
                                        in_=q_sb[0:D, :])
                                dq.dma_start(
                                    out=k_sb[32:32 + D, 2 * JT:],
                                    in_=k_sb[0:D, 2 * JT:])
                                for r in (2, 3):
                                    dq.dma_start(
                                        out=k_sb[32 * r:32 * r + D, JT:],
                                        in_=k_sb[0:D, JT:])
                            emit_vt_proj(g)
                        flush_av(av4)
                        pend_av.extend(new_av)
                        if g == 1:
                            emit_epilogue_step(0)
                        elif g == 3:
                            emit_epilogue_step(1)
                    flush_av(av4)
                    pend_epi.append([av4, js, {}])
                # final j-tile epilogue
                emit_epilogue_step(0)
                emit_epilogue_step(1)

            if loop_n:
                hints = (mybir.EngineType.PE, mybir.EngineType.Activation,
                         mybir.EngineType.DVE, mybir.EngineType.SP,
                         mybir.EngineType.Pool)
                with tc.For_i(0, loop_n, 1, hint_engines=hints):
                    for _ in range(bodies):
                        x_rounded[:] = [False] * 4
                        _compute()
            else:
                _compute()

    _fix_drain_waits(nc)
    return nc


_NC_CACHE = {}


def _get_nc():
    if "nc" not in _NC_CACHE:
        _NC_CACHE["nc"] = build_nc()
    return _NC_CACHE["nc"]


def kernel(**inputs) -> np.ndarray:
    x = np.ascontiguousarray(np.asarray(inputs["x"], dtype=np.float32))
    assert x.shape == (B, C, H, W), x.shape
    weights = {
        name: np.ascontiguousarray(np.asarray(inputs[name], dtype=np.float32))
        for name in ("Wq", "bq", "Wk", "bk", "Wv", "bv")
    }
    in_maps = [{"x": x[b].reshape(C, HW), **weights} for b in range(B)]
    nc = _get_nc()
    res = run_bass_kernel_spmd(nc, in_maps, core_ids=list(range(B)))
    out = np.stack([np.asarray(res.results[b]["out"]).reshape(C, H, W)
                    for b in range(B)])
    return out.astype(np.float32)
